# revision 14
# baseline (speedup 1.0000x reference)
"""Muscle-split variant: two independent recurrence chains (muscles 0-7 /
8-15), each N=256 columns, overlapping on the engines.  See kernel.py for
the base design; fx half-tile layout here is
  rows 0-63 e | 64-71 r-mask (DVE) | 72-74 w/rp/rt | 75-82 C-mask | 83-90 one-hot
K=91 per L1 matmul.
"""

import sys

import numpy as np

if "/opt/trn_rl_repo" not in sys.path:
    sys.path.insert(0, "/opt/trn_rl_repo")

B, T, E, H, M, NE = 256, 128, 64, 256, 16, 27
DT_SCALE = float(np.log1p(168.0))
NCORES = 8
BL = B // NCORES
N = BL * M                # 512
NH = N // 2               # 256 per half
GN = T * BL               # 4096


def _f32(x):
    return np.ascontiguousarray(np.asarray(x, dtype=np.float32))


def prepare_host(inputs):
    idx = np.asarray(inputs["exercise_idx"]).astype(np.int64)
    w = _f32(inputs["weight"]); rp = _f32(inputs["reps"])
    rt = _f32(inputs["rir_target"]); dt = _f32(inputs["delta_t"])
    ex_emb = _f32(inputs["ex_emb"]); mu_emb = _f32(inputs["mu_emb"])
    log_tau = _f32(inputs["log_tau"]); inv = _f32(inputs["involvement"])
    fW = [_f32(inputs[f"fW{i}"]) for i in (1, 2, 3, 4)]
    fb = [_f32(inputs[f"fb{i}"]) for i in (1, 2, 3, 4)]
    gW = [_f32(inputs[f"gW{i}"]) for i in (1, 2, 3, 4)]
    gb = [_f32(inputs[f"gb{i}"]) for i in (1, 2, 3, 4)]

    tau = np.exp(log_tau).astype(np.float32)
    dth = np.expm1(dt * np.float32(DT_SCALE)).astype(np.float32)
    A = np.exp(-dth[:, :, None] / tau[None, None, :]).astype(np.float32)
    A[:, 0, :] = 1.0
    C = (1.0 - A).astype(np.float32)
    inv_g = inv[idx]
    e_g = ex_emb[idx]

    mask8 = np.repeat(np.eye(8, dtype=np.float32), BL, axis=1)   # (8,256)

    fW1h = np.zeros((2, 91, H), dtype=np.float32)
    for hh in (0, 1):
        fW1h[hh, 0:64] = fW[0][4:68]
        fW1h[hh, 64:72] = np.tile(fW[0][3:4], (8, 1))
        fW1h[hh, 72] = fW[0][0]
        fW1h[hh, 73] = fW[0][1]
        fW1h[hh, 74] = fW[0][2]
        fW1h[hh, 75:83] = np.tile(fW[0][3:4], (8, 1))
        fW1h[hh, 83:91] = fb[0][None, :] + mu_emb[8 * hh:8 * hh + 8] @ fW[0][68:132]

    def pack2(Wm, cols):
        return np.concatenate([Wm[0:128], Wm[128:256]], axis=1).astype(np.float32)

    fW2p = pack2(fW[1], H); fW3p = pack2(fW[2], H // 2)
    fW4p = fW[3].reshape(128, 1)
    fbias = np.stack([fb[1][0:128], fb[1][128:256], fb[2]], axis=1)

    gW1a = np.zeros((66, H), dtype=np.float32)
    gW1a[0] = gW[0][0]; gW1a[1] = gW[0][1]; gW1a[2:66] = gW[0][2:66]
    gW1b = gW[0][66:82].reshape(2, 8, H).copy()
    gW2p = pack2(gW[1], H); gW3p = pack2(gW[2], H // 2)
    gW4p = gW[3].reshape(128, 1)
    gbias = np.stack(
        [gb[0][0:128], gb[0][128:256], gb[1][0:128], gb[1][128:256], gb[2]],
        axis=1)

    shared = dict(
        fW1h=fW1h, fW2p=fW2p, fW3p=fW3p, fW4p=fW4p, fbias=fbias,
        gW1a=gW1a, gW1b=gW1b, gW2p=gW2p, gW3p=gW3p, gW4p=gW4p, gbias=gbias,
        mask8=mask8,
    )
    scalars = dict(fb4=float(fb[3][0]), gb4=float(gb[3][0]))

    per_core = []
    for c in range(NCORES):
        S = slice(c * BL, (c + 1) * BL)
        eT = np.transpose(e_g[S], (1, 2, 0))                 # (T, E, BL)
        Cflat = np.transpose(C[S], (1, 2, 0)).reshape(T, N)  # m-major
        fxr = np.empty((T, 2, 75, NH), dtype=np.float32)
        for hh in (0, 1):
            fxr[:, hh, 0:64, :] = np.broadcast_to(
                eT[:, :, None, :], (T, E, 8, BL)).reshape(T, E, NH)
            fxr[:, hh, 64, :] = np.broadcast_to(
                w[S].T[:, None, :], (T, 8, BL)).reshape(T, NH)
            fxr[:, hh, 65, :] = np.broadcast_to(
                rp[S].T[:, None, :], (T, 8, BL)).reshape(T, NH)
            fxr[:, hh, 66, :] = np.broadcast_to(
                rt[S].T[:, None, :], (T, 8, BL)).reshape(T, NH)
            fxr[:, hh, 67:75, :] = (
                mask8[None, :, :] * Cflat[:, None, hh * NH:(hh + 1) * NH])

        aux = np.empty((T, 2, 8, 96), dtype=np.float32)
        At = np.transpose(A[S], (1, 2, 0))
        Ct = np.transpose(C[S], (1, 2, 0))
        It = np.transpose(inv_g[S], (1, 2, 0))
        for hh in (0, 1):
            mus = slice(8 * hh, 8 * hh + 8)
            aux[:, hh, :, 0:32] = At[:, mus]
            aux[:, hh, :, 32:64] = Ct[:, mus]
            aux[:, hh, :, 64:96] = It[:, mus]

        grows = np.empty((66, GN), dtype=np.float32)
        grows[0] = w[S].T.reshape(GN)
        grows[1] = rp[S].T.reshape(GN)
        grows[2:66] = np.transpose(e_g[S], (2, 1, 0)).reshape(E, GN)
        per_core.append(dict(fxrows=fxr, aux=aux, grows=grows))

    return shared, per_core, scalars


def build_program(scalars):
    import concourse.bass as bass
    import concourse.bacc as bacc
    import concourse.tile as tile
    from concourse import mybir
    from contextlib import ExitStack

    f32 = mybir.dt.float32
    f32r = mybir.dt.float32r
    AF = mybir.ActivationFunctionType
    OP = mybir.AluOpType

    nc = bacc.Bacc("TRN2", target_bir_lowering=False)

    d_in = {}
    for name, shape, dt_ in [
        ("fxrows", (T, 2, 75, NH), f32r), ("aux", (T, 2, 8, 96), f32),
        ("grows", (66, GN), f32r),
        ("fW1h", (2, 91, H), f32r), ("fW2p", (128, 2 * H), f32r),
        ("fW3p", (128, H), f32r), ("fW4p", (128, 1), f32r),
        ("fbias", (128, 3), f32),
        ("gW1a", (66, H), f32r), ("gW1b", (2, 8, H), f32r),
        ("gW2p", (128, 2 * H), f32r), ("gW3p", (128, H), f32r),
        ("gW4p", (128, 1), f32r), ("gbias", (128, 5), f32),
        ("mask8", (8, NH), f32r),
    ]:
        d_in[name] = nc.dram_tensor(name, list(shape), dt_, kind="ExternalInput").ap()
    d_rirs = nc.dram_tensor("rirs", [1, GN], f32, kind="ExternalOutput").ap()
    d_mpc = nc.dram_tensor("mpcout", [M, BL], f32, kind="ExternalOutput").ap()

    PB = 64

    with ExitStack() as ctx:
        tc = ctx.enter_context(tile.TileContext(nc))
        const = ctx.enter_context(tc.tile_pool(name="const", bufs=1))
        dmab = ctx.enter_context(tc.tile_pool(name="dmab", bufs=4))
        work = ctx.enter_context(tc.tile_pool(name="work", bufs=2))
        psum = ctx.enter_context(tc.tile_pool(name="psum", bufs=1, space="PSUM"))

        for cv, cn in [(0.0, "c0"), (scalars["fb4"], "cf4"),
                       (scalars["gb4"], "cg4")]:
            ct = const.tile([128, 1], f32, name=cn)
            nc.vector.memset(ct[:], cv)
            nc.const_aps.aps[(f32, float(cv))] = ct[:]

        WF1 = [const.tile([91, H], f32r, name=f"WF1_{h}") for h in (0, 1)]
        WF2 = const.tile([128, 2 * H], f32r)
        WF3 = const.tile([128, H], f32r)
        WF4 = const.tile([128, 1], f32r)
        FB = const.tile([128, 3], f32)
        GW1A = const.tile([66, H], f32r)
        GW1B = [const.tile([72, H], f32r, name=f"GW1B_{h}") for h in (0, 1)]
        GW2 = const.tile([128, 2 * H], f32r)
        GW3 = const.tile([128, H], f32r)
        GW4 = const.tile([128, 1], f32r)
        GB = const.tile([128, 5], f32)
        MASK = const.tile([72, NH], f32r)
        for t_, n_ in [(WF2, "fW2p"), (WF3, "fW3p"), (WF4, "fW4p"),
                       (FB, "fbias"), (GW1A, "gW1a"), (GW2, "gW2p"),
                       (GW3, "gW3p"), (GW4, "gW4p"), (GB, "gbias")]:
            nc.sync.dma_start(out=t_[:], in_=d_in[n_])
        for h in (0, 1):
            nc.sync.dma_start(out=WF1[h][:], in_=d_in["fW1h"][h])
            nc.sync.dma_start(out=GW1B[h][PB:PB + 8, :], in_=d_in["gW1b"][h])
        nc.sync.dma_start(out=MASK[PB:PB + 8, :], in_=d_in["mask8"])

        GS = [const.tile([72, GN], f32r, name=f"GS{h}") for h in (0, 1)]
        MPC = [const.tile([72, BL], f32, name=f"MPC{h}") for h in (0, 1)]
        for h in (0, 1):
            nc.vector.memset(MPC[h][PB:PB + 8, :], 1.0)

        fxs = [[const.tile([91, NH], f32r, tag=f"fx{h}_{i}", name=f"fx{h}_{i}")
                for i in range(3)] for h in (0, 1)]
        for h in (0, 1):
            for fx in fxs[h]:
                nc.sync.dma_start(out=fx[83:91, :], in_=d_in["mask8"])

        def bcast8(ap8x32):
            return bass.AP(
                tensor=ap8x32.tensor, offset=ap8x32.offset,
                ap=[list(ap8x32.ap[0]), [0, 8], list(ap8x32.ap[1])])

        s8 = lambda p: p[PB:PB + 8, :]
        f32v = lambda ap: ap.bitcast(f32)

        for t in range(T):
            for h in (0, 1):
                fx = fxs[h][t % 3]
                cs = slice(t * BL, (t + 1) * BL)
                nc.sync.dma_start(out=fx[0:64, :], in_=d_in["fxrows"][t, h, 0:64, :])
                nc.sync.dma_start(out=fx[72:83, :], in_=d_in["fxrows"][t, h, 64:75, :])
                aux = dmab.tile([72, 96], f32, tag=f"aux{h}", name=f"aux{h}")
                nc.sync.dma_start(out=aux[PB:PB + 8, :], in_=d_in["aux"][t, h])

                tmp = work.tile([72, BL], f32, tag=f"tmp{h}", name=f"tmp{h}")
                nc.vector.scalar_tensor_tensor(
                    out=s8(tmp), in0=s8(MPC[h]), scalar=0.1,
                    in1=aux[PB:PB + 8, 0:32], op0=OP.max, op1=OP.mult)
                nc.vector.tensor_tensor(
                    out=fx[64:72, :], in0=bcast8(s8(tmp)),
                    in1=f32v(s8(MASK)), op=OP.mult)
                nc.vector.tensor_tensor(
                    out=GS[h][PB:PB + 8, cs], in0=s8(tmp),
                    in1=aux[PB:PB + 8, 32:64], op=OP.add)

                p1 = psum.tile([128, 2 * NH], f32, tag=f"p1{h}", name=f"p1{h}")
                for mh in (0, 1):
                    nc.tensor.matmul(p1[:, mh * NH:(mh + 1) * NH],
                                     WF1[h][:, mh * 128:(mh + 1) * 128], fx[:],
                                     start=True, stop=True)
                h1 = work.tile([128, 2 * NH], f32r, tag=f"h1{h}", name=f"h1{h}")
                nc.scalar.activation(out=h1[:, 0:NH], in_=p1[:, 0:NH], func=AF.Relu)
                nc.vector.tensor_relu(out=h1[:, NH:2 * NH], in_=p1[:, NH:2 * NH])

                p2 = psum.tile([128, 2 * NH], f32, tag=f"p2{h}", name=f"p2{h}")
                for mh in (0, 1):
                    for k in (0, 1):
                        nc.tensor.matmul(
                            p2[:, mh * NH:(mh + 1) * NH],
                            WF2[:, k * H + mh * 128: k * H + mh * 128 + 128],
                            h1[:, k * NH:(k + 1) * NH],
                            start=(k == 0), stop=(k == 1))
                h2 = work.tile([128, 2 * NH], f32r, tag=f"h2{h}", name=f"h2{h}")
                nc.scalar.activation(out=h2[:, 0:NH], in_=p2[:, 0:NH],
                                     func=AF.Relu, bias=FB[:, 0:1])
                nc.vector.tensor_scalar(out=h2[:, NH:2 * NH], in0=p2[:, NH:2 * NH],
                                        scalar1=FB[:, 1:2], scalar2=0.0,
                                        op0=OP.add, op1=OP.max)

                p3 = psum.tile([128, NH], f32, tag=f"p3{h}", name=f"p3{h}")
                for k in (0, 1):
                    nc.tensor.matmul(p3[:], WF3[:, k * 128:(k + 1) * 128],
                                     h2[:, k * NH:(k + 1) * NH],
                                     start=(k == 0), stop=(k == 1))
                h3 = work.tile([128, NH], f32r, tag=f"h3{h}", name=f"h3{h}")
                nc.scalar.activation(out=h3[:], in_=p3[:], func=AF.Relu,
                                     bias=FB[:, 2:3])

                p4 = psum.tile([1, NH], f32, tag=f"p4{h}", name=f"p4{h}")
                nc.tensor.matmul(p4[:], WF4[:], h3[:], start=True, stop=True)
                df = work.tile([1, NH], f32, tag=f"df{h}", name=f"df{h}")
                nc.scalar.activation(out=df[:], in_=p4[:], func=AF.Sigmoid,
                                     bias=scalars["fb4"])
                dw = work.tile([72, BL], f32, tag=f"dw{h}", name=f"dw{h}")
                nc.sync.dma_start(out=dw[PB:PB + 8, :], in_=df[:])
                pt = work.tile([72, BL], f32, tag=f"pt{h}", name=f"pt{h}")
                nc.vector.tensor_tensor(out=s8(pt), in0=f32v(GS[h][PB:PB + 8, cs]),
                                        in1=aux[PB:PB + 8, 64:96], op=OP.mult)
                vt = work.tile([72, BL], f32, tag=f"vt{h}", name=f"vt{h}")
                nc.vector.tensor_tensor(out=s8(vt), in0=s8(pt), in1=s8(dw),
                                        op=OP.mult)
                nc.vector.scalar_tensor_tensor(
                    out=s8(MPC[h]), in0=s8(vt), scalar=-1.0,
                    in1=f32v(GS[h][PB:PB + 8, cs]), op0=OP.mult, op1=OP.add)

        # ---- batched g-MLP
        GX = const.tile([66, GN], f32r)
        nc.sync.dma_start(out=GX[:], in_=d_in["grows"])
        RS = const.tile([1, GN], f32)
        for chk in range(GN // N):
            cs = slice(chk * N, (chk + 1) * N)
            g1 = work.tile([128, 2 * N], f32r, tag="g1", name="g1")
            for mh in (0, 1):
                q1 = psum.tile([128, N], f32, tag=f"p1{mh}", name=f"q1{mh}")
                nc.tensor.matmul(q1[:],
                                 GW1A[:, mh * 128:(mh + 1) * 128], GX[:, cs],
                                 start=True, stop=False)
                nc.tensor.matmul(q1[:],
                                 GW1B[0][PB:PB + 8, mh * 128:(mh + 1) * 128],
                                 GS[0][PB:PB + 8, cs], start=False, stop=False)
                nc.tensor.matmul(q1[:],
                                 GW1B[1][PB:PB + 8, mh * 128:(mh + 1) * 128],
                                 GS[1][PB:PB + 8, cs], start=False, stop=True)
                if mh == 0:
                    nc.scalar.activation(out=g1[:, 0:N], in_=q1[:], func=AF.Relu,
                                         bias=GB[:, 0:1])
                else:
                    nc.vector.tensor_scalar(out=g1[:, N:2 * N], in0=q1[:],
                                            scalar1=GB[:, 1:2], scalar2=0.0,
                                            op0=OP.add, op1=OP.max)
            g2 = work.tile([128, 2 * N], f32r, tag="g2", name="g2")
            for mh in (0, 1):
                q2 = psum.tile([128, N], f32, tag=f"p2{mh}", name=f"q2{mh}")
                for k in (0, 1):
                    nc.tensor.matmul(
                        q2[:],
                        GW2[:, k * H + mh * 128: k * H + mh * 128 + 128],
                        g1[:, k * N:(k + 1) * N],
                        start=(k == 0), stop=(k == 1))
                if mh == 0:
                    nc.scalar.activation(out=g2[:, 0:N], in_=q2[:], func=AF.Relu,
                                         bias=GB[:, 2:3])
                else:
                    nc.vector.tensor_scalar(out=g2[:, N:2 * N], in0=q2[:],
                                            scalar1=GB[:, 3:4], scalar2=0.0,
                                            op0=OP.add, op1=OP.max)
            q3 = psum.tile([128, N], f32, tag="p30", name="q3")
            for k in (0, 1):
                nc.tensor.matmul(q3[:], GW3[:, k * 128:(k + 1) * 128],
                                 g2[:, k * N:(k + 1) * N],
                                 start=(k == 0), stop=(k == 1))
            g3 = work.tile([128, N], f32r, tag="g3", name="g3")
            nc.scalar.activation(out=g3[:], in_=q3[:], func=AF.Relu,
                                 bias=GB[:, 4:5])
            q4 = psum.tile([1, N], f32, tag="p40", name="q4")
            nc.tensor.matmul(q4[:], GW4[:], g3[:], start=True, stop=True)
            nc.scalar.activation(out=RS[0:1, cs], in_=q4[:], func=AF.Sigmoid,
                                 bias=scalars["gb4"])

        nc.sync.dma_start(out=d_rirs, in_=RS[:])
        mpcf = const.tile([72, BL], f32)
        for h in (0, 1):
            nc.vector.tensor_scalar_max(mpcf[PB:PB + 8, :], MPC[h][PB:PB + 8, :], 0.1)
            nc.sync.dma_start(out=d_mpc[8 * h:8 * h + 8, :],
                              in_=mpcf[PB:PB + 8, :])

    nc.compile()
    return nc


def kernel(**inputs):
    from concourse.bass_utils import run_bass_kernel_spmd

    shared, per_core, scalars = prepare_host(inputs)
    nc = build_program(scalars)
    in_maps = [{**shared, **pc} for pc in per_core]
    res = run_bass_kernel_spmd(nc, in_maps, list(range(NCORES))).results

    rirs = np.empty((B, T), dtype=np.float32)
    mpc = np.empty((B, M), dtype=np.float32)
    for c in range(NCORES):
        S = slice(c * BL, (c + 1) * BL)
        rirs[S] = res[c]["rirs"].reshape(T, BL).T
        mpc[S] = res[c]["mpcout"].reshape(M, BL).T
    return rirs, mpc
